# revision 1
# baseline (speedup 1.0000x reference)
"""ConvShapeletNet Trainium2 kernel.

Math (per batch row b, channel c):
  xb = x.reshape(B, C, L)                    # pure view: row r=(b,c) is 8192 contiguous floats
  win[o]  = sum(xb[r, o*286 : o*286+1146])   # o in [0, 25)
  y       = (win + conv_bias[c])^2
  pooled  = max(-y over window 3)  = -(min y over window 3)   -> (B, 10, 8)
  out     = pooled.reshape(B, 80) @ fc_w.T + fc_b

Window sum decomposition: 1146 = 4*286 + 2, so with 286-block sums L2[k]:
  win[o] = L2[o] + L2[o+1] + L2[o+2] + L2[o+3] + x[o*286+1144] + x[o*286+1145]

Sharding: pure data parallel, batch 512 -> 64 per core across 8 cores.
"""

import numpy as np
from contextlib import ExitStack

import concourse.bass as bass
import concourse.tile as tile
from concourse import bacc, masks, mybir
from concourse.bass_utils import run_bass_kernel_spmd

F32 = mybir.dt.float32

N_CORES = 8
B_FULL = 512
B_SH = B_FULL // N_CORES     # 64 batches per core
C = 10                       # variates / conv groups
L = 8192
ROWS = B_SH * C              # 640 rows of 8192 per core
BAG = 1146
STRIDE = 286
L_OUT = 25
NBLK = 28                    # 286-blocks used (27+3 <= 28 <= 8192//286)
L_P = 8
POOLK = 3
N_CLASSES = 10
TILE_P = 128
N_TILES = ROWS // TILE_P     # 5


def build_nc(reps=1, x_bufs=5, strided_out=False):
    """Build the per-core program. reps>1 unrolls the whole computation
    multiple times inside one NEFF (identical result; used for timing).
    strided_out=True restores the old (b, n)-layout output DMA (A/B only)."""
    nc = bacc.Bacc("TRN2", target_bir_lowering=False, debug=False,
                   num_devices=N_CORES)

    x = nc.dram_tensor("x", [ROWS, L], F32, kind="ExternalInput")
    conv_bias = nc.dram_tensor("conv_bias", [C], F32, kind="ExternalInput")
    fc_w = nc.dram_tensor("fc_w", [N_CLASSES, C * L_P], F32, kind="ExternalInput")
    fc_b = nc.dram_tensor("fc_b", [N_CLASSES], F32, kind="ExternalInput")
    # out is stored transposed (n, b): the final DMA is then 10 contiguous
    # 256B lines instead of 640 strided 4B segments (descriptor-bound on HW).
    # The host transposes during unshard.
    out_shape = [B_SH, N_CLASSES] if strided_out else [N_CLASSES, B_SH]
    out = nc.dram_tensor("out", out_shape, F32, kind="ExternalOutput")
    # scratch for replicating conv_bias so the per-tile bias column is an
    # affine gather: rep[j] = conv_bias[j % 10]  ->  btab[p, t] = rep[128*t + p]
    rep = nc.dram_tensor("bias_rep", [TILE_P * C], F32)

    with tile.TileContext(nc) as tc, ExitStack() as ctx:
        const = ctx.enter_context(tc.tile_pool(name="const", bufs=1))
        xpool = ctx.enter_context(tc.tile_pool(name="x", bufs=x_bufs))
        work = ctx.enter_context(tc.tile_pool(name="work", bufs=2))
        # one buffer per tile: no WAR reuse waits land on the pool min-reduce
        # (TensorReduce descriptors allow a single sync wait)
        pooledp = ctx.enter_context(tc.tile_pool(name="pooledp", bufs=N_TILES))
        tpsum = ctx.enter_context(
            tc.tile_pool(name="tpsum", bufs=min(N_TILES, 5), space="PSUM"))
        opsum = ctx.enter_context(tc.tile_pool(name="opsum", bufs=2, space="PSUM"))
        mtp = ctx.enter_context(tc.tile_pool(name="mtp", bufs=2))

        # ---- constants (once) ----
        ident = const.tile([TILE_P, TILE_P], F32)
        masks.make_identity(nc, ident[:])

        # w8[k, n*10+c] = fc_w[n, c*8+k], built without any gather DMA:
        # load fc_w contiguously (10 lines x 320B), then one tiny PE
        # transpose per channel c: fc_w[:, 8c:8c+8] (10, 8) -> (8, 10) [k, n],
        # copied into the strided w8 column slice. A direct DMA of this
        # layout would be 800 4-byte descriptors.
        fw = const.tile([N_CLASSES, C * L_P], F32)
        nc.sync.dma_start(out=fw[:], in_=fc_w.ap())
        w8 = const.tile([L_P, N_CLASSES * C], F32)
        w8v3 = w8[:].rearrange("k (n c) -> k n c", c=C)

        fcb = const.tile([N_CLASSES, 1], F32)
        nc.sync.dma_start(out=fcb[:], in_=fc_b.ap().unsqueeze(1))

        # bias broadcast: conv_bias (10,) -> bb (128, 10) -> rep DRAM -> btab (128, 5)
        bb = const.tile([TILE_P, C], F32)
        nc.sync.dma_start(out=bb[:],
                          in_=conv_bias.ap().unsqueeze(0).broadcast_to((TILE_P, C)))
        nc.sync.dma_start(out=rep.ap().rearrange("(p c) -> p c", c=C), in_=bb[:])
        btab = const.tile([TILE_P, N_TILES], F32)
        nc.sync.dma_start(
            out=btab[:],
            in_=rep.ap()[0:TILE_P * N_TILES].rearrange("(t p) -> p t", p=TILE_P))

        wps = ctx.enter_context(tc.tile_pool(name="wps", bufs=1, space="PSUM"))
        for c in range(C):
            wt = wps.tile([L_P, N_CLASSES], F32, tag="wt")
            nc.tensor.transpose(wt[:], fw[:, c * L_P:(c + 1) * L_P],
                                ident[0:N_CLASSES, 0:N_CLASSES])
            nc.scalar.copy(w8v3[:, :, c], wt[:])

        xap = x.ap()
        # DMA chunks aligned to 7-block (2002-element) groups: TensorReduce's
        # HW descriptor supports only ONE sync wait, so each reduce (and each
        # strided-extras add) must depend on exactly one DMA. 2002 = 7*286
        # also aligns the extras pairs at 1144+286*o to chunk boundaries.
        CH = 7 * STRIDE                       # 2002
        bounds = [0, CH, 2 * CH, 3 * CH, L]   # last chunk [6006:8192)
        # extras window groups (o-range, owning chunk): pairs at 1144+286*o
        exgrp = [(0, 3), (3, 10), (10, 17), (17, 24), (24, 25)]

        for _ in range(reps):
            # transposed pooled accumulator: mt[k, r] = pooled[r, k]
            mt = mtp.tile([L_P, ROWS], F32, tag="mt")
            for t in range(N_TILES):
                xt = xpool.tile([TILE_P, L], F32, tag="xt")
                nc.sync.dma_start(out=xt[:],
                                  in_=xap[t * TILE_P:(t + 1) * TILE_P, :])

                # 286-block sums; each reduce reads exactly one DMA chunk
                l2 = work.tile([TILE_P, NBLK], F32, tag="l2")
                for q in range(7):
                    nc.vector.reduce_sum(
                        l2[:, q * 4:(q + 1) * 4],
                        xt[:, q * 4 * STRIDE:(q + 1) * 4 * STRIDE].rearrange(
                            "p (k j) -> p k j", j=STRIDE),
                        axis=mybir.AxisListType.X)

                # win[o] = L2[o]+L2[o+1]+L2[o+2]+L2[o+3] + x[o*286+1144] + x[o*286+1145]
                t1 = work.tile([TILE_P, L_OUT], F32, tag="t1")
                nc.gpsimd.tensor_add(t1[:], l2[:, 0:25], l2[:, 1:26])
                t2 = work.tile([TILE_P, L_OUT], F32, tag="t2")
                nc.gpsimd.tensor_add(t2[:], l2[:, 2:27], l2[:, 3:28])
                win = work.tile([TILE_P, L_OUT], F32, tag="win")
                nc.gpsimd.tensor_add(win[:], t1[:], t2[:])

                # strided extras in chunk-aligned groups (one DMA dep each),
                # on the otherwise-idle GPSIMD engine: the xt buffer is then
                # released at max(DVE reduces, GPSIMD extras) instead of after
                # a longer serial DVE chain (-1.9us/body measured on HW)
                ex = work.tile([TILE_P, L_OUT], F32, tag="ex")
                xv = xt[:, 1144:1144 + 24 * STRIDE].rearrange(
                    "p (o j) -> p o j", j=STRIDE)
                nc.gpsimd.tensor_add(ex[:, 0:24], xv[:, :, 0], xv[:, :, 1])
                nc.gpsimd.tensor_add(ex[:, 24:25], xt[:, 8008:8009],
                                     xt[:, 8009:8010])
                nc.gpsimd.tensor_add(win[:], win[:], ex[:])

                # y = (win + bias)^2 on ScalarE; pooled = -(min_3 y) = max_3(-y)
                y = work.tile([TILE_P, L_OUT], F32, tag="y")
                nc.scalar.activation(y[:], win[:],
                                     mybir.ActivationFunctionType.Square,
                                     bias=btab[:, t:t + 1], scale=1.0)
                pooled = pooledp.tile([TILE_P, L_P], F32, tag="pooled")
                nc.vector.tensor_reduce(
                    pooled[:],
                    y[:, 0:L_P * POOLK].rearrange("p (k j) -> p k j", j=POOLK),
                    axis=mybir.AxisListType.X, op=mybir.AluOpType.min, negate=True)

                # PE transpose (128, 8) -> (8, 128), stash into mt
                pt = tpsum.tile([L_P, TILE_P], F32, tag="pt")
                nc.tensor.transpose(pt[:], pooled[:], ident[:])
                nc.scalar.copy(mt[:, t * TILE_P:(t + 1) * TILE_P], pt[:])

            # FC: out[n, b] = sum_c sum_k w8[k, n*10+c] * mt[k, b*10+c]
            ops = opsum.tile([N_CLASSES, B_SH], F32, tag="ops")
            mtv = mt[:].rearrange("k (b c) -> k b c", c=C)
            w8v = w8[:].rearrange("k (n c) -> k n c", c=C)
            for c in range(C):
                nc.tensor.matmul(ops[:], w8v[:, :, c],
                                 mtv[:, :, c], start=(c == 0), stop=(c == C - 1))
            outsb = mtp.tile([N_CLASSES, B_SH], F32, tag="outsb")
            nc.scalar.add(outsb[:], ops[:], fcb[:, 0:1])
            nc.sync.dma_start(
                out=out.ap().transpose([1, 0]) if strided_out else out.ap(),
                in_=outsb[:])

    nc.compile()
    return nc


_NC_CACHE = None


def _get_nc():
    global _NC_CACHE
    if _NC_CACHE is None:
        _NC_CACHE = build_nc()
    return _NC_CACHE


def make_in_maps(x, conv_bias, fc_w, fc_b):
    x = np.ascontiguousarray(np.asarray(x, dtype=np.float32))
    conv_bias = np.asarray(conv_bias, dtype=np.float32)
    fc_w = np.asarray(fc_w, dtype=np.float32)
    fc_b = np.asarray(fc_b, dtype=np.float32)
    in_maps = []
    for i in range(N_CORES):
        shard = x[i * B_SH:(i + 1) * B_SH]          # (64, 8192, 10)
        in_maps.append({
            "x": shard.reshape(ROWS, L),            # the reference's view reshape
            "conv_bias": conv_bias,
            "fc_w": fc_w,
            "fc_b": fc_b,
        })
    return in_maps


def kernel(x, conv_bias, fc_w, fc_b, trace=False):
    nc = _get_nc()
    in_maps = make_in_maps(x, conv_bias, fc_w, fc_b)
    res = run_bass_kernel_spmd(nc, in_maps, list(range(N_CORES)), trace=trace)
    kernel.last_result = res
    # per-core output is (n_classes, batch_shard): transpose while unsharding
    out = np.concatenate([res.results[i]["out"].T for i in range(N_CORES)], axis=0)
    return np.ascontiguousarray(out, dtype=np.float32)



# revision 2
# speedup vs baseline: 1.4437x; 1.4437x over previous
"""ConvShapeletNet Trainium2 kernel — bf16-staged, fold-tree block sums.

Math (per batch row b, channel c):
  xb = x.reshape(B, C, L)                    # pure view: row r=(b,c) is 8192 contiguous floats
  win[o]  = sum(xb[r, o*286 : o*286+1146])   # o in [0, 24): only 24 of 25 conv outputs
                                             # survive MaxPool1d(3) (floor(25/3)*3 = 24)
  y       = (win + conv_bias[c])^2
  pooled  = max(-y over window 3) = -(min y over window 3)    -> (B, 10, 8)
  out     = pooled.reshape(B, 80) @ fc_w.T + fc_b

Only x[:, 0:7724] is ever read (23*286 + 1146 = 7724), and the 2e-2 rel-err
budget allows bf16 input staging: the host packs x into 27 zero-padded
288-element blocks (+2 tail extras) of bf16. That halves HBM traffic — the
sole bottleneck — and the 288 = 2^5 * 9 padding keeps every fold-tree slice
4-byte aligned so DVE tensor_tensor runs in 2x_1p mode (tensor_reduce is
capped at 1x, so a reduce-only block sum would be 2x slower than DMA).

Per 128-row tile:
  l2[k]  = sum(block k)        k in [0, 27): 4 bf16 fold adds (288->18) + one
                               18-wide f32 tensor_reduce
  ex[o]  = x[286(o+4)] + x[286(o+4)+1]  (pairs sit at padded-block starts;
                               one [128, 24, 2] -> [128, 24] f32 reduce)
  win[o] = l2[o]+l2[o+1]+l2[o+2]+l2[o+3] + ex[o]   (f32 adds on idle GPSIMD)

Sharding: pure data parallel, batch 512 -> 64 per core across 8 cores.
"""

import numpy as np
from contextlib import ExitStack

import ml_dtypes
import concourse.bass as bass
import concourse.tile as tile
from concourse import bacc, masks, mybir
from concourse.bass_utils import run_bass_kernel_spmd

F32 = mybir.dt.float32
BF16 = mybir.dt.bfloat16
NP_BF16 = ml_dtypes.bfloat16

N_CORES = 8
B_FULL = 512
B_SH = B_FULL // N_CORES     # 64 batches per core
C = 10                       # variates / conv groups
L = 8192
ROWS = B_SH * C              # 640 rows per core
STRIDE = 286
NB = 27                      # 286-blocks summed (win[o] needs blocks o..o+3, o<24)
BLK = 288                    # padded block width (2^5 * 9: alignment-clean folds)
XT_W = 28 * BLK              # 8064: padded staging row (28th block = 2 tail extras)
L_X = NB * BLK + 2           # 7778 columns actually DMA'd
L_OUT = 24                   # windows that survive pooling
L_P = 8
POOLK = 3
N_CLASSES = 10
TILE_P = 128
N_TILES = ROWS // TILE_P     # 5


def build_nc(reps=1, x_bufs=6, strided_out=False):
    """Build the per-core program. reps>1 unrolls the whole computation
    multiple times inside one NEFF (identical result; used for timing)."""
    nc = bacc.Bacc("TRN2", target_bir_lowering=False, debug=False,
                   num_devices=N_CORES)

    x = nc.dram_tensor("x", [ROWS, XT_W], BF16, kind="ExternalInput")
    conv_bias = nc.dram_tensor("conv_bias", [C], F32, kind="ExternalInput")
    fc_w = nc.dram_tensor("fc_w", [N_CLASSES, C * L_P], F32, kind="ExternalInput")
    fc_b = nc.dram_tensor("fc_b", [N_CLASSES], F32, kind="ExternalInput")
    # out is stored transposed (n, b): the final DMA is then 10 contiguous
    # 256B lines instead of 640 strided 4B segments (descriptor-bound on HW).
    # The host transposes during unshard.
    out_shape = [B_SH, N_CLASSES] if strided_out else [N_CLASSES, B_SH]
    out = nc.dram_tensor("out", out_shape, F32, kind="ExternalOutput")
    # scratch for replicating conv_bias so the per-tile bias column is an
    # affine gather: rep[j] = conv_bias[j % 10]  ->  btab[p, t] = rep[128*t + p]
    rep = nc.dram_tensor("bias_rep", [TILE_P * C], F32)

    with tile.TileContext(nc) as tc, ExitStack() as ctx:
        const = ctx.enter_context(tc.tile_pool(name="const", bufs=1))
        xpool = ctx.enter_context(tc.tile_pool(name="x", bufs=x_bufs))
        work = ctx.enter_context(tc.tile_pool(name="work", bufs=2))
        pooledp = ctx.enter_context(tc.tile_pool(name="pooledp", bufs=N_TILES))
        tpsum = ctx.enter_context(
            tc.tile_pool(name="tpsum", bufs=min(N_TILES, 5), space="PSUM"))
        opsum = ctx.enter_context(tc.tile_pool(name="opsum", bufs=2, space="PSUM"))
        mtp = ctx.enter_context(tc.tile_pool(name="mtp", bufs=2))

        # ---- constants (once) ----
        ident = const.tile([TILE_P, TILE_P], F32)
        masks.make_identity(nc, ident[:])

        # w8[k, n*10+c] = fc_w[n, c*8+k], built without any gather DMA:
        # load fc_w contiguously, then one tiny PE transpose per channel.
        fw = const.tile([N_CLASSES, C * L_P], F32)
        nc.sync.dma_start(out=fw[:], in_=fc_w.ap())
        w8 = const.tile([L_P, N_CLASSES * C], F32)
        w8v3 = w8[:].rearrange("k (n c) -> k n c", c=C)

        fcb = const.tile([N_CLASSES, 1], F32)
        nc.sync.dma_start(out=fcb[:], in_=fc_b.ap().unsqueeze(1))

        # bias broadcast: conv_bias (10,) -> bb (128, 10) -> rep DRAM -> btab (128, 5)
        bb = const.tile([TILE_P, C], F32)
        nc.sync.dma_start(out=bb[:],
                          in_=conv_bias.ap().unsqueeze(0).broadcast_to((TILE_P, C)))
        nc.sync.dma_start(out=rep.ap().rearrange("(p c) -> p c", c=C), in_=bb[:])
        btab = const.tile([TILE_P, N_TILES], F32)
        nc.sync.dma_start(
            out=btab[:],
            in_=rep.ap()[0:TILE_P * N_TILES].rearrange("(t p) -> p t", p=TILE_P))

        wps = ctx.enter_context(tc.tile_pool(name="wps", bufs=1, space="PSUM"))
        for c in range(C):
            wt = wps.tile([L_P, N_CLASSES], F32, tag="wt")
            nc.tensor.transpose(wt[:], fw[:, c * L_P:(c + 1) * L_P],
                                ident[0:N_CLASSES, 0:N_CLASSES])
            nc.scalar.copy(w8v3[:, :, c], wt[:])

        xap = x.ap()
        for _ in range(reps):
            # transposed pooled accumulator: mt[k, r] = pooled[r, k]
            mt = mtp.tile([L_P, ROWS], F32, tag="mt")
            for t in range(N_TILES):
                xt = xpool.tile([TILE_P, XT_W], BF16, tag="xt")
                nc.sync.dma_start(out=xt[:, 0:L_X],
                                  in_=xap[t * TILE_P:(t + 1) * TILE_P, 0:L_X])
                xv = xt[:].rearrange("p (k j) -> p k j", j=BLK)  # [128, 28, 288]

                # bf16 fold tree 288 -> 144 -> 72 -> 36 -> 18 (2x_1p mode),
                # then one 18-wide 1x reduce into f32 block sums
                fb1 = work.tile([TILE_P, NB * 144], BF16, tag="fb1")
                f1 = fb1[:].rearrange("p (k j) -> p k j", j=144)
                nc.vector.tensor_add(f1, xv[:, 0:NB, 0:144], xv[:, 0:NB, 144:288])
                fb2 = work.tile([TILE_P, NB * 72], BF16, tag="fb2")
                f2 = fb2[:].rearrange("p (k j) -> p k j", j=72)
                nc.vector.tensor_add(f2, f1[:, :, 0:72], f1[:, :, 72:144])
                fb3 = work.tile([TILE_P, NB * 36], BF16, tag="fb3")
                f3 = fb3[:].rearrange("p (k j) -> p k j", j=36)
                nc.vector.tensor_add(f3, f2[:, :, 0:36], f2[:, :, 36:72])
                fb4 = work.tile([TILE_P, NB * 18], BF16, tag="fb4")
                f4 = fb4[:].rearrange("p (k j) -> p k j", j=18)
                nc.vector.tensor_add(f4, f3[:, :, 0:18], f3[:, :, 18:36])
                l2 = work.tile([TILE_P, NB], F32, tag="l2")
                nc.vector.reduce_sum(l2[:], f4, axis=mybir.AxisListType.X)

                # extras: first two elements of padded blocks 4..27
                ex = work.tile([TILE_P, L_OUT], F32, tag="ex")
                nc.vector.reduce_sum(ex[:], xv[:, 4:28, 0:2],
                                     axis=mybir.AxisListType.X)

                # win[o] = l2[o]+l2[o+1]+l2[o+2]+l2[o+3] + ex[o] on idle GPSIMD
                t1 = work.tile([TILE_P, L_OUT], F32, tag="t1")
                nc.gpsimd.tensor_add(t1[:], l2[:, 0:24], l2[:, 1:25])
                t2 = work.tile([TILE_P, L_OUT], F32, tag="t2")
                nc.gpsimd.tensor_add(t2[:], l2[:, 2:26], l2[:, 3:27])
                t3 = work.tile([TILE_P, L_OUT], F32, tag="t3")
                nc.gpsimd.tensor_add(t3[:], t1[:], t2[:])
                win = work.tile([TILE_P, L_OUT], F32, tag="win")
                nc.gpsimd.tensor_add(win[:], t3[:], ex[:])

                # y = (win + bias)^2 on ScalarE; pooled = -(min_3 y) = max_3(-y)
                y = work.tile([TILE_P, L_OUT], F32, tag="y")
                nc.scalar.activation(y[:], win[:],
                                     mybir.ActivationFunctionType.Square,
                                     bias=btab[:, t:t + 1], scale=1.0)
                pooled = pooledp.tile([TILE_P, L_P], F32, tag="pooled")
                nc.vector.tensor_reduce(
                    pooled[:],
                    y[:].rearrange("p (k j) -> p k j", j=POOLK),
                    axis=mybir.AxisListType.X, op=mybir.AluOpType.min, negate=True)

                # PE transpose (128, 8) -> (8, 128), stash into mt
                pt = tpsum.tile([L_P, TILE_P], F32, tag="pt")
                nc.tensor.transpose(pt[:], pooled[:], ident[:])
                nc.scalar.copy(mt[:, t * TILE_P:(t + 1) * TILE_P], pt[:])

            # FC: out[n, b] = sum_c sum_k w8[k, n*10+c] * mt[k, b*10+c]
            ops = opsum.tile([N_CLASSES, B_SH], F32, tag="ops")
            mtv = mt[:].rearrange("k (b c) -> k b c", c=C)
            w8v = w8[:].rearrange("k (n c) -> k n c", c=C)
            for c in range(C):
                nc.tensor.matmul(ops[:], w8v[:, :, c],
                                 mtv[:, :, c], start=(c == 0), stop=(c == C - 1))
            outsb = mtp.tile([N_CLASSES, B_SH], F32, tag="outsb")
            nc.scalar.add(outsb[:], ops[:], fcb[:, 0:1])
            nc.sync.dma_start(
                out=out.ap().transpose([1, 0]) if strided_out else out.ap(),
                in_=outsb[:])

    nc.compile()
    return nc


_NC_CACHE = None


def _get_nc():
    global _NC_CACHE
    if _NC_CACHE is None:
        _NC_CACHE = build_nc()
    return _NC_CACHE


def _stage_x(shard):
    """(B_SH, 8192, 10) f32 -> (640, 8064) bf16, 288-padded blocks."""
    xr = np.ascontiguousarray(shard).reshape(ROWS, L)
    xp = np.zeros((ROWS, 28, BLK), dtype=NP_BF16)
    xp[:, 0:NB, 0:STRIDE] = xr[:, 0:NB * STRIDE].reshape(ROWS, NB, STRIDE)
    xp[:, NB, 0:2] = xr[:, NB * STRIDE:NB * STRIDE + 2]
    return xp.reshape(ROWS, XT_W)


def make_in_maps(x, conv_bias, fc_w, fc_b):
    x = np.asarray(x, dtype=np.float32)
    conv_bias = np.asarray(conv_bias, dtype=np.float32)
    fc_w = np.asarray(fc_w, dtype=np.float32)
    fc_b = np.asarray(fc_b, dtype=np.float32)
    in_maps = []
    for i in range(N_CORES):
        in_maps.append({
            "x": _stage_x(x[i * B_SH:(i + 1) * B_SH]),
            "conv_bias": conv_bias,
            "fc_w": fc_w,
            "fc_b": fc_b,
        })
    return in_maps


def kernel(x, conv_bias, fc_w, fc_b, trace=False):
    nc = _get_nc()
    in_maps = make_in_maps(x, conv_bias, fc_w, fc_b)
    res = run_bass_kernel_spmd(nc, in_maps, list(range(N_CORES)), trace=trace)
    kernel.last_result = res
    # per-core output is (n_classes, batch_shard): transpose while unsharding
    out = np.concatenate([res.results[i]["out"].T for i in range(N_CORES)], axis=0)
    return np.ascontiguousarray(out, dtype=np.float32)


# revision 3
# speedup vs baseline: 1.7025x; 1.1792x over previous
"""ConvShapeletNet Trainium2 kernel — fp16 fold-order staging.

Math (per batch row b, channel c):
  xb = x.reshape(B, C, L)                    # pure view: row r=(b,c) is 8192 contiguous floats
  win[o]  = sum(xb[r, o*286 : o*286+1146])   # o in [0, 24): only 24 of 25 conv outputs
                                             # survive MaxPool1d(3) (floor(25/3)*3 = 24)
  y       = (win + conv_bias[c])^2
  pooled  = max(-y over window 3) = -(min y over window 3)    -> (B, 10, 8)
  out     = pooled.reshape(B, 80) @ fc_w.T + fc_b

Only x[:, 0:7724] is ever read (23*286 + 1146 = 7724), and the 2e-2 rel-err
budget allows fp16 input staging — halving HBM traffic, the sole bottleneck.

Block sums: win[o] = L2[o]+L2[o+1]+L2[o+2]+L2[o+3] + x[286(o+4)] + x[286(o+4)+1]
with L2[k] = sum(x[286k : 286k+286]), k in [0, 27).

DVE tensor_reduce is capped at 1x mode (1 elem/cycle/partition), but
tensor_tensor add runs 2x_1p on dense 16-bit data. So the host pre-permutes
each zero-padded 288-element block into fold order — element j = a0*144 +
a1*72 + a2*36 + a3*18 + r goes to lane (a0,a1,a2,a3) slot (k, r) — making
every fold level a single CONTIGUOUS [128, N] + [128, N] add (the two halves
of the previous level's output). Four 2x folds (7776 -> 486) + one 18-wide
1x reduce gives L2 in f32. The 24 extras pairs are staged as a 48-column
appendix so they too reduce from a contiguous view.

Sharding: pure data parallel, batch 512 -> 64 per core across 8 cores.
"""

import numpy as np
from contextlib import ExitStack

import concourse.bass as bass
import concourse.tile as tile
from concourse import bacc, masks, mybir
from concourse.bass_utils import run_bass_kernel_spmd

F32 = mybir.dt.float32
F16 = mybir.dt.float16

N_CORES = 8
B_FULL = 512
B_SH = B_FULL // N_CORES     # 64 batches per core
C = 10                       # variates / conv groups
L = 8192
ROWS = B_SH * C              # 640 rows per core
STRIDE = 286
NB = 27                      # 286-blocks summed (win[o] needs blocks o..o+3, o<24)
BLK = 288                    # zero-padded block (16 fold lanes x 18)
NR = 18                      # residual reduce width after 4 folds
MAIN_W = 16 * NB * NR        # 7776 fold-ordered columns
EX_OFF = MAIN_W
N_EX = 24                    # extras pairs (one per window)
XT_W = MAIN_W + 2 * N_EX     # 7824 staged columns per row
L_OUT = 24                   # windows that survive pooling
L_P = 8
POOLK = 3
N_CLASSES = 10
TILE_P = 128
N_TILES = ROWS // TILE_P     # 5


def build_nc(reps=1, x_bufs=6, strided_out=False):
    """Build the per-core program. reps>1 unrolls the whole computation
    multiple times inside one NEFF (identical result; used for timing)."""
    nc = bacc.Bacc("TRN2", target_bir_lowering=False, debug=False,
                   num_devices=N_CORES)

    x = nc.dram_tensor("x", [ROWS, XT_W], F16, kind="ExternalInput")
    conv_bias = nc.dram_tensor("conv_bias", [C], F32, kind="ExternalInput")
    fc_w = nc.dram_tensor("fc_w", [N_CLASSES, C * L_P], F32, kind="ExternalInput")
    fc_b = nc.dram_tensor("fc_b", [N_CLASSES], F32, kind="ExternalInput")
    # out is stored transposed (n, b): the final DMA is then 10 contiguous
    # 256B lines instead of 640 strided 4B segments (descriptor-bound on HW).
    # The host transposes during unshard.
    out_shape = [B_SH, N_CLASSES] if strided_out else [N_CLASSES, B_SH]
    out = nc.dram_tensor("out", out_shape, F32, kind="ExternalOutput")
    # scratch for replicating conv_bias so the per-tile bias column is an
    # affine gather: rep[j] = conv_bias[j % 10]  ->  btab[p, t] = rep[128*t + p]
    rep = nc.dram_tensor("bias_rep", [TILE_P * C], F32)

    with tile.TileContext(nc) as tc, ExitStack() as ctx:
        const = ctx.enter_context(tc.tile_pool(name="const", bufs=1))
        xpool = ctx.enter_context(tc.tile_pool(name="x", bufs=x_bufs))
        work = ctx.enter_context(tc.tile_pool(name="work", bufs=2))
        pooledp = ctx.enter_context(tc.tile_pool(name="pooledp", bufs=N_TILES))
        tpsum = ctx.enter_context(
            tc.tile_pool(name="tpsum", bufs=min(N_TILES, 5), space="PSUM"))
        opsum = ctx.enter_context(tc.tile_pool(name="opsum", bufs=2, space="PSUM"))
        mtp = ctx.enter_context(tc.tile_pool(name="mtp", bufs=2))

        # ---- constants (once) ----
        ident = const.tile([TILE_P, TILE_P], F32)
        masks.make_identity(nc, ident[:])

        # w8[k, n*10+c] = fc_w[n, c*8+k], built without any gather DMA:
        # load fc_w contiguously, then one tiny PE transpose per channel.
        fw = const.tile([N_CLASSES, C * L_P], F32)
        nc.sync.dma_start(out=fw[:], in_=fc_w.ap())
        w8 = const.tile([L_P, N_CLASSES * C], F32)
        w8v3 = w8[:].rearrange("k (n c) -> k n c", c=C)

        fcb = const.tile([N_CLASSES, 1], F32)
        nc.sync.dma_start(out=fcb[:], in_=fc_b.ap().unsqueeze(1))

        # bias broadcast: conv_bias (10,) -> bb (128, 10) -> rep DRAM -> btab (128, 5)
        bb = const.tile([TILE_P, C], F32)
        nc.sync.dma_start(out=bb[:],
                          in_=conv_bias.ap().unsqueeze(0).broadcast_to((TILE_P, C)))
        nc.sync.dma_start(out=rep.ap().rearrange("(p c) -> p c", c=C), in_=bb[:])
        btab = const.tile([TILE_P, N_TILES], F32)
        nc.sync.dma_start(
            out=btab[:],
            in_=rep.ap()[0:TILE_P * N_TILES].rearrange("(t p) -> p t", p=TILE_P))

        wps = ctx.enter_context(tc.tile_pool(name="wps", bufs=1, space="PSUM"))
        for c in range(C):
            wt = wps.tile([L_P, N_CLASSES], F32, tag="wt")
            nc.tensor.transpose(wt[:], fw[:, c * L_P:(c + 1) * L_P],
                                ident[0:N_CLASSES, 0:N_CLASSES])
            nc.scalar.copy(w8v3[:, :, c], wt[:])

        xap = x.ap()
        for _ in range(reps):
            # transposed pooled accumulator: mt[k, r] = pooled[r, k]
            mt = mtp.tile([L_P, ROWS], F32, tag="mt")
            for t in range(N_TILES):
                xt = xpool.tile([TILE_P, XT_W], F16, tag="xt")
                nc.sync.dma_start(out=xt[:],
                                  in_=xap[t * TILE_P:(t + 1) * TILE_P, :])

                # fp16 fold tree, each level a contiguous halves-add (2x_1p):
                # 7776 -> 3888 -> 1944 -> 972 -> 486
                fb1 = work.tile([TILE_P, 8 * NB * NR], F16, tag="fb1")
                nc.vector.tensor_add(fb1[:], xt[:, 0:3888], xt[:, 3888:7776])
                fb2 = work.tile([TILE_P, 4 * NB * NR], F16, tag="fb2")
                nc.vector.tensor_add(fb2[:], fb1[:, 0:1944], fb1[:, 1944:3888])
                fb3 = work.tile([TILE_P, 2 * NB * NR], F16, tag="fb3")
                nc.vector.tensor_add(fb3[:], fb2[:, 0:972], fb2[:, 972:1944])
                fb4 = work.tile([TILE_P, NB * NR], F16, tag="fb4")
                nc.vector.tensor_add(fb4[:], fb3[:, 0:486], fb3[:, 486:972])
                l2 = work.tile([TILE_P, NB], F32, tag="l2")
                nc.vector.reduce_sum(
                    l2[:], fb4[:].rearrange("p (k j) -> p k j", j=NR),
                    axis=mybir.AxisListType.X)

                # extras appendix: ex[o] = x[286(o+4)] + x[286(o+4)+1]
                ex = work.tile([TILE_P, L_OUT], F32, tag="ex")
                nc.vector.reduce_sum(
                    ex[:], xt[:, EX_OFF:XT_W].rearrange("p (o j) -> p o j", j=2),
                    axis=mybir.AxisListType.X)

                # win[o] = l2[o]+l2[o+1]+l2[o+2]+l2[o+3] + ex[o] on idle GPSIMD
                t1 = work.tile([TILE_P, L_OUT], F32, tag="t1")
                nc.gpsimd.tensor_add(t1[:], l2[:, 0:24], l2[:, 1:25])
                t2 = work.tile([TILE_P, L_OUT], F32, tag="t2")
                nc.gpsimd.tensor_add(t2[:], l2[:, 2:26], l2[:, 3:27])
                t3 = work.tile([TILE_P, L_OUT], F32, tag="t3")
                nc.gpsimd.tensor_add(t3[:], t1[:], t2[:])
                win = work.tile([TILE_P, L_OUT], F32, tag="win")
                nc.gpsimd.tensor_add(win[:], t3[:], ex[:])

                # y = (win + bias)^2 on ScalarE; pooled = -(min_3 y) = max_3(-y)
                y = work.tile([TILE_P, L_OUT], F32, tag="y")
                nc.scalar.activation(y[:], win[:],
                                     mybir.ActivationFunctionType.Square,
                                     bias=btab[:, t:t + 1], scale=1.0)
                pooled = pooledp.tile([TILE_P, L_P], F32, tag="pooled")
                nc.vector.tensor_reduce(
                    pooled[:],
                    y[:].rearrange("p (k j) -> p k j", j=POOLK),
                    axis=mybir.AxisListType.X, op=mybir.AluOpType.min, negate=True)

                # PE transpose (128, 8) -> (8, 128), stash into mt
                pt = tpsum.tile([L_P, TILE_P], F32, tag="pt")
                nc.tensor.transpose(pt[:], pooled[:], ident[:])
                nc.scalar.copy(mt[:, t * TILE_P:(t + 1) * TILE_P], pt[:])

            # FC: out[n, b] = sum_c sum_k w8[k, n*10+c] * mt[k, b*10+c]
            ops = opsum.tile([N_CLASSES, B_SH], F32, tag="ops")
            mtv = mt[:].rearrange("k (b c) -> k b c", c=C)
            w8v = w8[:].rearrange("k (n c) -> k n c", c=C)
            for c in range(C):
                nc.tensor.matmul(ops[:], w8v[:, :, c],
                                 mtv[:, :, c], start=(c == 0), stop=(c == C - 1))
            outsb = mtp.tile([N_CLASSES, B_SH], F32, tag="outsb")
            nc.scalar.add(outsb[:], ops[:], fcb[:, 0:1])
            nc.sync.dma_start(
                out=out.ap().transpose([1, 0]) if strided_out else out.ap(),
                in_=outsb[:])

    nc.compile()
    return nc


_NC_CACHE = None


def _get_nc():
    global _NC_CACHE
    if _NC_CACHE is None:
        _NC_CACHE = build_nc()
    return _NC_CACHE


# extras column indices in the original row: x[286(o+4)], x[286(o+4)+1]
_EX_IDX = np.add.outer(STRIDE * (np.arange(N_EX) + 4), np.arange(2)).ravel()


def _stage_x(shard):
    """(B_SH, 8192, 10) f32 -> (640, 7824) fp16 in fold order.

    Element j = a0*144 + a1*72 + a2*36 + a3*18 + r of padded block k lands at
    column ((((a0*2+a1)*2+a2)*2+a3)*27 + k)*18 + r, so each fold level's two
    operands are the contiguous halves of the previous level. Columns
    7776:7824 hold the 24 extras pairs.
    """
    xr = np.ascontiguousarray(shard).reshape(ROWS, L)
    blocks = np.zeros((ROWS, NB, BLK), dtype=np.float16)
    blocks[:, :, 0:STRIDE] = xr[:, 0:NB * STRIDE].reshape(ROWS, NB, STRIDE)
    blocks[:, NB - 1, STRIDE:STRIDE + 2] = 0  # (286/287 already zero; explicit)
    # (ROWS, k, a0, a1, a2, a3, r) -> (ROWS, a0, a1, a2, a3, k, r)
    perm = blocks.reshape(ROWS, NB, 2, 2, 2, 2, NR).transpose(0, 2, 3, 4, 5, 1, 6)
    out = np.empty((ROWS, XT_W), dtype=np.float16)
    out[:, 0:MAIN_W] = perm.reshape(ROWS, MAIN_W)
    out[:, EX_OFF:XT_W] = xr[:, _EX_IDX]
    return out


def make_in_maps(x, conv_bias, fc_w, fc_b):
    x = np.asarray(x, dtype=np.float32)
    conv_bias = np.asarray(conv_bias, dtype=np.float32)
    fc_w = np.asarray(fc_w, dtype=np.float32)
    fc_b = np.asarray(fc_b, dtype=np.float32)
    in_maps = []
    for i in range(N_CORES):
        in_maps.append({
            "x": _stage_x(x[i * B_SH:(i + 1) * B_SH]),
            "conv_bias": conv_bias,
            "fc_w": fc_w,
            "fc_b": fc_b,
        })
    return in_maps


def kernel(x, conv_bias, fc_w, fc_b, trace=False):
    nc = _get_nc()
    in_maps = make_in_maps(x, conv_bias, fc_w, fc_b)
    res = run_bass_kernel_spmd(nc, in_maps, list(range(N_CORES)), trace=trace)
    kernel.last_result = res
    # per-core output is (n_classes, batch_shard): transpose while unsharding
    out = np.concatenate([res.results[i]["out"].T for i in range(N_CORES)], axis=0)
    return np.ascontiguousarray(out, dtype=np.float32)
